# revision 24
# baseline (speedup 1.0000x reference)
"""3x3 valid cross-correlation of 64 1024x1024 f32 images on 8 TRN2 NeuronCores.

Strategy: pure data-parallel over batch (8 images per core). All HBM traffic
rides in bf16: the host casts the f32 input to bf16 (rel err ~2.9e-3, far
under the 2e-2 gate), the device computes bf16 matmuls with f32 PSUM
accumulation, PSUM->SBUF copies cast back down to bf16, and the host upcasts
the stored bf16 result. That halves both DMA directions vs f32 (per-core
traffic 67 MB -> 33.6 MB; HBM roofline ~94 us at 358 GB/s per core).

DMA layout: the host pre-packs the input into the exact SBUF tile layout
x[img, p, b, c] = img_row[126*b + p, c] so per-image loads are fully
contiguous quarter-DMAs (512 KiB, 4 KiB per partition, 2 KiB aligned); the
2-row overlap between consecutive 128-row blocks is materialized on the
host. The output is stored in the symmetric packed layout with rows padded
from 1022 to 1024 elements (2 KiB aligned), in four parts as blocks
complete, and the host un-permutes and strips the padding.

Each image is processed as 8 full row-blocks (128 input rows -> 126 output
rows) on the TensorEngine: 3 PSUM-accumulated matmuls per 512-wide column
segment, where a banded [128, 128] bf16 stationary (padded to 128 columns
so Fast Weight Load kicks in) applies the 3 vertical taps of kernel column
dj and the moving operand is the image block column-shifted by dj (free-dim
AP offset). The last 14 output rows of ALL 8 images are computed together
in one 128-partition pass: images are packed two per 32-partition group
(block-diagonal [32, 28] stationary) and the four groups run concurrently
on the PE's 32x32 sub-arrays via tile_position, writing one PSUM tile that
is staged and stored with a single 256 KiB DMA. Input DMAs ride the SP
HWDGE ring, output DMAs the ACT ring, so loads and stores interleave.
"""

import numpy as np
import ml_dtypes

import concourse.bacc as bacc
import concourse.mybir as mybir
from concourse.tile import TileContext

B = 64          # batch
D = 1024        # image side
O = D - 2       # 1022 output side
N_CORES = 8
BPC = B // N_CORES  # images per core
BLK = 126       # output rows per full block
NBLK = 8        # full blocks per image
TAIL_M = O - 8 * BLK   # 14 tail output rows
TAIL_K = 16     # tail input rows (1008..1023)

_F32 = mybir.dt.float32
_BF16 = mybir.dt.bfloat16
_NP_BF16 = ml_dtypes.bfloat16


def _make_bands(ker):
    """Banded stationary matrices from the 3x3 kernel.

    A[k, dj, m] = ker[k-m, dj]  (k-m in 0..2) -> 126 output rows per block;
    columns 126..127 are zero padding (full 128-wide stationary enables FWL).
    T2 packs two images per 32-partition group, block-diagonally:
      T2[32j + k,      dj, m]      = ker[k-m, dj]   (image 2j,   m < 14)
      T2[32j + 16 + k, dj, 14 + m] = ker[k-m, dj]   (image 2j+1)
    """
    A = np.zeros((128, 3, 128), np.float32)
    T2 = np.zeros((128, 3, 2 * TAIL_M), np.float32)
    for dj in range(3):
        for di in range(3):
            A[np.arange(BLK) + di, dj, np.arange(BLK)] = ker[di, dj]
            for j in range(4):
                T2[32 * j + np.arange(TAIL_M) + di, dj, np.arange(TAIL_M)] = (
                    ker[di, dj]
                )
                T2[
                    32 * j + 16 + np.arange(TAIL_M) + di,
                    dj,
                    TAIL_M + np.arange(TAIL_M),
                ] = ker[di, dj]
    return A.astype(_NP_BF16), T2.astype(_NP_BF16)


def _build(loop_iters=None, mode="full"):
    """Build the per-core Bass program. loop_iters wraps the whole workload
    in a For_i loop (benchmarking variant; kernel() uses loop_iters=None).
    mode: "full" | "dma" (loads+stores only) | "pe" (compute only)."""
    nc = bacc.Bacc()
    x = nc.dram_tensor("x", [BPC, 128, 8, D], _BF16, kind="ExternalInput")
    xt = nc.dram_tensor("xt", [128, D], _BF16, kind="ExternalInput")
    bandA = nc.dram_tensor("bandA", [128, 3, 128], _BF16, kind="ExternalInput")
    bandT = nc.dram_tensor(
        "bandT", [128, 3, 2 * TAIL_M], _BF16, kind="ExternalInput"
    )
    y = nc.dram_tensor("y", [BPC, BLK, 8, D], _BF16, kind="ExternalOutput")
    yt = nc.dram_tensor("yt", [128, D], _BF16, kind="ExternalOutput")

    with TileContext(nc) as tc:
        with (
            tc.tile_pool(name="bands", bufs=1) as bands,
            tc.tile_pool(name="xin", bufs=6) as xin,
            tc.tile_pool(name="xtail", bufs=1) as xtail,
            tc.tile_pool(name="ps", bufs=4, space="PSUM") as ps,
            tc.tile_pool(name="yout", bufs=4) as yout,
            tc.tile_pool(name="ytst", bufs=1) as ytst,
        ):
            A = bands.tile([128, 3, 128], _BF16)
            T2 = bands.tile([128, 3, 2 * TAIL_M], _BF16)
            nc.sync.dma_start(A[:], bandA[:])
            nc.sync.dma_start(T2[:], bandT[:])
            if mode == "pe":
                X0 = xin.tile([128, 8, D], _BF16, tag="x0")
                nc.sync.dma_start(X0[:], x[0])

            def tails():
                """All 8 images' last-14 output rows in one 128-partition
                pass: 4 concurrent 32x32 tile groups, 2 images per group."""
                XT = xtail.tile([128, D], _BF16, tag="xt")
                if mode != "pe":
                    nc.sync.dma_start(XT[:], xt[:])
                else:
                    nc.vector.tensor_copy(XT[:], X0[:, 0, :])
                if mode == "dma":
                    nc.scalar.dma_start(yt[:], XT[:])
                    return
                PG = ps.tile([128, O], _F32, tag="p")
                for s0, sl in ((0, 512), (512, 510)):
                    for dj in range(3):
                        for j in range(4):
                            nc.tensor.matmul(
                                PG[32 * j : 32 * j + 2 * TAIL_M, s0 : s0 + sl],
                                lhsT=T2[32 * j : 32 * j + 32, dj, :],
                                rhs=XT[32 * j : 32 * j + 32, dj + s0 : dj + s0 + sl],
                                start=(dj == 0),
                                stop=(dj == 2),
                                tile_position=(32 * j, 32 * j),
                            )
                ST = ytst.tile([128, D], _BF16, tag="st")
                nc.vector.tensor_copy(ST[:, :O], PG[:, :])
                if mode != "pe":
                    nc.scalar.dma_start(yt[:], ST[:])

            def one_image(img):
                if mode == "pe":
                    X = X0
                else:
                    X = xin.tile([128, 8, D], _BF16, tag="x")
                    # Image 0 loads in quarters so the first matmuls start
                    # ~1.6 us after the iteration barrier; later images load
                    # whole (2 MiB) — large transfers keep the SP ring at
                    # full HBM rate (fine-grained DMAs cost ~20% bandwidth).
                    if img == 0:
                        for q in range(4):
                            nc.sync.dma_start(
                                X[:, 2 * q : 2 * q + 2, :],
                                x[img, :, 2 * q : 2 * q + 2, :],
                            )
                    else:
                        nc.sync.dma_start(X[:], x[img])

                if mode == "dma":
                    # store the loaded tile straight back: pure DMA A/B probe
                    nc.scalar.dma_start(y[img], X[:BLK, 0:8, :])
                    return

                Y = yout.tile([128, NBLK, D], _BF16, tag="y")
                for b in range(NBLK):
                    P = ps.tile([128, O], _F32, tag="p")
                    for s0, sl in ((0, 512), (512, 510)):
                        for dj in range(3):
                            nc.tensor.matmul(
                                P[:, s0 : s0 + sl],
                                lhsT=A[:, dj, :],
                                rhs=X[:, b, dj + s0 : dj + s0 + sl],
                                start=(dj == 0),
                                stop=(dj == 2),
                            )
                    # PSUM evacuation all on DVE so the ACT engine (store
                    # ring) never queues a store behind copy work.
                    nc.vector.tensor_copy(Y[:BLK, b, :O], P[:BLK, :])
                    # Last image stores in pairs (512 KiB) so the iteration
                    # drain only waits on the final ~0.5 MiB; other images
                    # store whole (2 MiB) for full HBM rate on the ACT ring.
                    if mode != "pe" and img == BPC - 1 and b in (1, 3, 5, 7):
                        nc.scalar.dma_start(
                            y[img, :, b - 1 : b + 1, :], Y[:BLK, b - 1 : b + 1, :]
                        )
                if mode != "pe" and img != BPC - 1:
                    nc.scalar.dma_start(y[img], Y[:BLK, 0:8, :])

            def all_images():
                tails()
                for img in range(BPC):
                    one_image(img)

            if loop_iters is None:
                all_images()
            else:
                with tc.For_i(0, loop_iters, 1):
                    all_images()
    nc.compile()
    return nc


_CACHE = {}


def _make_runner(nc, donate=True):
    """Wrap a finalized Bass program in a jitted SPMD runner.

    Mirrors run_bass_via_pjrt: operands are (inputs..., zero outputs...,
    partition-id), in exactly the jit parameter order neuronx_cc_hook
    requires.
    """
    import jax
    from jax.sharding import Mesh, PartitionSpec
    from jax.experimental.shard_map import shard_map
    from concourse.bass2jax import (
        _bass_exec_p,
        partition_id_tensor,
        install_neuronx_cc_hook,
    )

    install_neuronx_cc_hook()
    partition_name = nc.partition_id_tensor.name if nc.partition_id_tensor else None

    in_names, out_names, out_avals, zero_outs = [], [], [], []
    for alloc in nc.m.functions[0].allocations:
        if not isinstance(alloc, mybir.MemoryLocationSet):
            continue
        name = alloc.memorylocations[0].name
        if alloc.kind == "ExternalInput":
            if name != partition_name:
                in_names.append(name)
        elif alloc.kind == "ExternalOutput":
            out_names.append(name)
            shape = tuple(alloc.tensor_shape)
            dtype = mybir.dt.np(alloc.dtype)
            out_avals.append(jax.core.ShapedArray(shape, dtype))
            zero_outs.append(np.zeros(shape, dtype))
    n_params = len(in_names)
    n_outs = len(out_avals)
    all_names = in_names + out_names
    if partition_name is not None:
        all_names.append(partition_name)

    def _body(*args):
        outs = _bass_exec_p.bind(
            *args,
            partition_id_tensor(),
            out_avals=tuple(out_avals),
            in_names=tuple(all_names),
            out_names=tuple(out_names),
            lowering_input_output_aliases=(),
            sim_require_finite=True,
            sim_require_nnan=True,
            nc=nc,
        )
        return tuple(outs)

    devices = jax.devices()[:N_CORES]
    mesh = Mesh(np.asarray(devices), ("core",))
    fn = jax.jit(
        shard_map(
            _body,
            mesh=mesh,
            in_specs=(PartitionSpec("core"),) * (n_params + n_outs),
            out_specs=(PartitionSpec("core"),) * n_outs,
            check_rep=False,
        ),
        donate_argnums=(
            tuple(range(n_params, n_params + n_outs)) if donate else ()
        ),
        keep_unused=True,
    )
    return fn, in_names, out_names, zero_outs


def _get_runner(loop_iters=None, donate=True, mode="full"):
    key = ("runner", loop_iters, donate, mode)
    if key not in _CACHE:
        _CACHE[key] = _make_runner(_build(loop_iters, mode=mode), donate=donate)
    return _CACHE[key]


def _concat_inputs(inputs, ker):
    A, T2 = _make_bands(np.asarray(ker, np.float32).reshape(3, 3))
    xb = np.asarray(inputs, np.float32).astype(_NP_BF16).reshape(B, D, D)
    es = xb.strides[-1]  # element stride (2 bytes)
    # x[img, p, b, c] = xb[img, 126*b + p, c]  (2-row overlap materialized)
    xv = np.lib.stride_tricks.as_strided(
        xb, shape=(B, 8, 128, D),
        strides=(xb.strides[0], BLK * D * es, D * es, es),
    )
    x = np.ascontiguousarray(xv.transpose(0, 2, 1, 3))
    # xt[core, 32j + 16*i2 + r, :] = tail row r of image (8*core + 2j + i2)
    xt = (
        xb[:, D - TAIL_K :, :]
        .reshape(N_CORES, 4, 2, TAIL_K, D)
        .transpose(0, 1, 2, 3, 4)
        .reshape(N_CORES * 128, D)
    )
    xt = np.ascontiguousarray(xt)
    return {
        "x": x,
        "xt": xt,
        "bandA": np.ascontiguousarray(
            np.broadcast_to(A, (N_CORES,) + A.shape)
        ).reshape(N_CORES * 128, 3, 128),
        "bandT": np.ascontiguousarray(
            np.broadcast_to(T2, (N_CORES,) + T2.shape)
        ).reshape(N_CORES * 128, 3, 2 * TAIL_M),
    }


def kernel(inputs, kernel):
    import jax

    fn, in_names, out_names, zero_outs = _get_runner()
    concat = _concat_inputs(inputs, kernel)
    zeros = [
        np.zeros((N_CORES * z.shape[0], *z.shape[1:]), z.dtype) for z in zero_outs
    ]
    outs = fn(*[concat[n] for n in in_names], *zeros)
    outs = jax.block_until_ready(outs)
    res = {n: np.asarray(o) for n, o in zip(out_names, outs)}
    # un-permute: y[img, 126*b + p, c] = y_dev[img, p, b, c]; strip pad cols
    yf = np.empty((B, O, O), np.float32)
    ym = res["y"].transpose(0, 2, 1, 3)[:, :, :, :O].astype(np.float32)
    yf[:, : 8 * BLK, :] = ym.reshape(B, 8 * BLK, O)
    # tails: yt[core, 32j + m] = image (8c+2j) tail row m (m<14),
    #        yt[core, 32j + 14 + m] = image (8c+2j+1) tail row m
    yt = res["yt"].reshape(N_CORES, 4, 32, D)[:, :, :, :O].astype(np.float32)
    tails = yt[:, :, : 2 * TAIL_M, :].reshape(N_CORES, 4, 2, TAIL_M, O)
    yf[:, 8 * BLK :, :] = tails.reshape(B, TAIL_M, O)
    return yf.reshape(B, O * O)
